# revision 10
# baseline (speedup 1.0000x reference)
"""NTM read controller kernel for Trainium2 (8 NeuronCores, SPMD data-parallel over batch).

Problem shapes (hardcoded): B=64, E=512, N=8192, M=64, Dense out = M+6 = 70.
Sharding: batch 64 -> 8 cores x 8 batches. Each core is fully independent.

Per-core layout: n = 64*p + j  (partition p in [0,128), j in [0,64)).
memory_weights[b] loads as a [128, 4096] SBUF slab with 16 KiB contiguous per
partition (full-rate DMA).  Engine split:
  - DVE: dot-product mul + grouped reduces + small chain ops
  - ACT: square pass, Exp/Ln/Sqrt/Softplus/Sigmoid (scale = per-partition AP)
  - PE : controller matmul, partition-sum broadcast (all-ones matmul),
         circular-shift boundary columns (permutation matmul), weighted read
"""

import sys

for _p in ("/opt/trn_rl_repo", "/root/.axon_site/_ro/trn_rl_repo"):
    if _p not in sys.path:
        sys.path.insert(0, _p)

import numpy as np

import concourse.bass as bass
import concourse.bacc as bacc
import concourse.mybir as mybir
from concourse.tile import TileContext

F32 = mybir.dt.float32
AF = mybir.ActivationFunctionType
ALU = mybir.AluOpType
AX = mybir.AxisListType

B_LOC = 8      # batches per core
E = 512
N = 8192
M = 64
C_OUT = 70     # M + 6
P = 128        # partitions
J = 64         # n = 64*p + j
EPS = 1e-8

_NC_CACHE = None


def _build_nc():
    nc = bacc.Bacc("TRN2", target_bir_lowering=False, debug=False, num_devices=8)

    emb_d = nc.dram_tensor("embeddings", [B_LOC, E], F32, kind="ExternalInput")
    wp_d = nc.dram_tensor("w_prev", [B_LOC, N], F32, kind="ExternalInput")
    mem_d = nc.dram_tensor("memory_weights", [B_LOC, N, M], F32, kind="ExternalInput")
    W_d = nc.dram_tensor("W", [E, C_OUT], F32, kind="ExternalInput")
    b_d = nc.dram_tensor("b", [C_OUT], F32, kind="ExternalInput")
    md_d = nc.dram_tensor("memory_data", [B_LOC, M], F32, kind="ExternalOutput")
    wout_d = nc.dram_tensor("w_out", [B_LOC, N], F32, kind="ExternalOutput")

    with TileContext(nc) as tc:
        with (
            tc.tile_pool(name="const", bufs=1) as cpool,
            tc.tile_pool(name="slab", bufs=2) as spool,
            tc.tile_pool(name="small", bufs=2) as smpool,
            tc.tile_pool(name="pscratch", bufs=2, space="PSUM") as ppool,
            tc.tile_pool(name="pmd", bufs=2, space="PSUM") as pmd,
        ):
            # ---------------- constants ----------------
            ones128 = cpool.tile([P, P], F32)
            nc.vector.memset(ones128, 1.0)

            id8 = cpool.tile([8, 8], F32)
            nc.vector.memset(id8, 1.0)
            nc.gpsimd.affine_select(
                id8, id8, pattern=[[-1, 8]], compare_op=ALU.is_equal,
                fill=0.0, base=0, channel_multiplier=1,
            )

            # roll(+1): out[p] = in[(p-1) mod 128]  -> Sdn[p, j]=1 iff j=p+1, patch [127,0]
            sdn = cpool.tile([P, P], F32)
            nc.vector.memset(sdn, 1.0)
            nc.gpsimd.affine_select(
                sdn, sdn, pattern=[[-1, P]], compare_op=ALU.is_equal,
                fill=0.0, base=1, channel_multiplier=1,
            )
            # wrap element (127, 0) via a second mask + add (engine APs must
            # start at partition 0/32/64/96, so no direct offset memset)
            sdn_wrap = cpool.tile([P, P], F32)
            nc.vector.memset(sdn_wrap, 1.0)
            nc.gpsimd.affine_select(
                sdn_wrap, sdn_wrap, pattern=[[-1, P]], compare_op=ALU.is_equal,
                fill=0.0, base=-(P - 1), channel_multiplier=1,
            )
            nc.vector.tensor_tensor(sdn, sdn, sdn_wrap, ALU.add)

            # roll(-1): out[p] = in[(p+1) mod 128]  -> Sup[p, j]=1 iff j=p-1, patch [0,127]
            sup = cpool.tile([P, P], F32)
            nc.vector.memset(sup, 1.0)
            nc.gpsimd.affine_select(
                sup, sup, pattern=[[-1, P]], compare_op=ALU.is_equal,
                fill=0.0, base=-1, channel_multiplier=1,
            )
            nc.vector.memset(sup[0:1, P - 1 : P], 1.0)

            ones18 = cpool.tile([1, 8], F32)
            nc.vector.memset(ones18, 1.0)

            # ---------------- small inputs ----------------
            W4 = cpool.tile([P, 4 * C_OUT], F32)
            nc.sync.dma_start(
                out=W4.rearrange("p (c j) -> p c j", c=4),
                in_=W_d.rearrange("(c p) j -> p c j", p=P),
            )
            b_sb = cpool.tile([1, C_OUT], F32)
            nc.sync.dma_start(out=b_sb, in_=b_d.rearrange("(a j) -> a j", a=1))

            emb_sb = cpool.tile([B_LOC, E], F32)
            nc.sync.dma_start(out=emb_sb, in_=emb_d[:, :])

            wp_slab = cpool.tile([P, B_LOC * J], F32)
            nc.sync.dma_start(
                out=wp_slab.rearrange("p (b j) -> p b j", b=B_LOC),
                in_=wp_d.rearrange("b (p j) -> p b j", p=P),
            )

            w_slab = cpool.tile([P, B_LOC * J], F32)   # final w output staging
            md_row = cpool.tile([1, B_LOC * M], F32)   # final memory_data staging

            # ---------------- controller: addr = emb @ W + b ----------------
            eT_sb = cpool.tile([P, 32], F32)  # 4 chunks of emb^T [128, 8]
            for c in range(4):
                eT_ps = ppool.tile([P, 8], F32, name=f"eT_ps{c}", tag="scratch")
                nc.tensor.matmul(
                    eT_ps, emb_sb[:, c * P : (c + 1) * P], id8, is_transpose=True
                )
                nc.vector.tensor_copy(eT_sb[:, c * 8 : (c + 1) * 8], eT_ps)

            addr_ps = ppool.tile([B_LOC, C_OUT], F32, tag="scratch")
            for c in range(4):
                nc.tensor.matmul(
                    addr_ps,
                    eT_sb[:, c * 8 : (c + 1) * 8],
                    W4.rearrange("p (c j) -> p c j", c=4)[:, c, :],
                    start=(c == 0),
                    stop=False,
                )
            nc.tensor.matmul(addr_ps, ones18, b_sb, start=False, stop=True)
            addr_sb = cpool.tile([B_LOC, C_OUT], F32)
            nc.vector.tensor_copy(addr_sb, addr_ps)

            # ---------------- controller nonlinearities ----------------
            # ctrl columns: 0=beta 1=g 2=1-g 3=y 4=k_norm 5..7=s
            ctrl = cpool.tile([B_LOC, 8], F32)
            sp_t = cpool.tile([B_LOC, 2], F32)  # softplus scratch: [beta_raw, y_raw]
            nc.scalar.activation(sp_t[:, 0:1], addr_sb[:, 64:65], AF.Exp)
            nc.scalar.activation(sp_t[:, 1:2], addr_sb[:, 69:70], AF.Exp)
            nc.vector.tensor_scalar(sp_t, sp_t, 1.0, None, ALU.add)
            nc.scalar.activation(ctrl[:, 0:1], sp_t[:, 0:1], AF.Ln)       # beta
            nc.scalar.activation(ctrl[:, 3:4], sp_t[:, 1:2], AF.Ln)      # softplus(y_raw)
            nc.scalar.activation(ctrl[:, 1:2], addr_sb[:, 65:66], AF.Sigmoid)
            nc.vector.tensor_scalar(
                ctrl[:, 2:3], ctrl[:, 1:2], -1.0, 1.0, ALU.mult, ALU.add
            )
            nc.vector.tensor_scalar(ctrl[:, 3:4], ctrl[:, 3:4], 1.0, None, ALU.add)

            ksq = cpool.tile([B_LOC, M], F32)
            knsq = cpool.tile([B_LOC, 1], F32)
            nc.scalar.activation(ksq, addr_sb[:, 0:M], AF.Square, accum_out=knsq)
            nc.scalar.activation(ctrl[:, 4:5], knsq, AF.Sqrt)

            sexp = cpool.tile([B_LOC, 3], F32)
            ssum = cpool.tile([B_LOC, 1], F32)
            nc.scalar.activation(sexp, addr_sb[:, 66:69], AF.Exp, accum_out=ssum)
            srec = cpool.tile([B_LOC, 1], F32)
            nc.vector.reciprocal(srec, ssum)
            nc.vector.tensor_scalar_mul(ctrl[:, 5:8], sexp, srec)

            # ---------------- per-batch broadcast of controller scalars ----------------
            cb_list = []
            kb_list = []
            ones8p = cpool.tile([B_LOC, P], F32)
            nc.vector.memset(ones8p, 1.0)
            for b in range(B_LOC):
                # sel[p, :] = 1 iff p == b  (row-select mask, built from partition 0)
                sel = cpool.tile([B_LOC, P], F32, name=f"sel{b}")
                nc.gpsimd.affine_select(
                    sel, ones8p, pattern=[[0, P]], compare_op=ALU.is_equal,
                    fill=0.0, base=-b, channel_multiplier=1,
                )

                cb_ps = ppool.tile([P, 8], F32, name=f"cb_ps{b}", tag="scratch")
                nc.tensor.matmul(cb_ps, sel, ctrl)
                cb = cpool.tile([P, 8], F32, name=f"cb{b}")
                nc.vector.tensor_copy(cb, cb_ps)
                cb_list.append(cb)

                kb_ps = ppool.tile([P, M], F32, name=f"kb_ps{b}", tag="scratch")
                nc.tensor.matmul(kb_ps, sel, addr_sb[:, 0:M])
                kb = cpool.tile([P, M], F32, name=f"kb{b}")
                nc.vector.tensor_copy(kb, kb_ps)
                kb_list.append(kb)

            # ---------------- main per-batch loop ----------------
            for b in range(B_LOC):
                cb = cb_list[b]
                beta_bc = cb[:, 0:1]
                g_bc = cb[:, 1:2]
                omg_bc = cb[:, 2:3]
                y_bc = cb[:, 3:4]
                kn_bc = cb[:, 4:5]
                s0_bc = cb[:, 5:6]
                s1_bc = cb[:, 6:7]
                s2_bc = cb[:, 7:8]
                kb = kb_list[b]

                mem_sb = spool.tile([P, J * M], F32, name=f"mem{b}", tag="mem")
                nc.sync.dma_start(
                    out=mem_sb,
                    in_=mem_d[b].rearrange("(p j) m -> p (j m)", p=P),
                )
                mem3 = mem_sb.rearrange("p (j m) -> p j m", m=M)

                # dot[p, j] = sum_m mem[p, j, m] * k[m]
                prod = spool.tile([P, J * M], F32, name=f"prod{b}", tag="prod")
                kb3 = kb.unsqueeze(1).broadcast_to([P, J, M])
                nc.vector.tensor_tensor(
                    prod.rearrange("p (j m) -> p j m", m=M), mem3, kb3, ALU.mult
                )
                dot = smpool.tile([P, J], F32, name=f"dot{b}", tag="dot")
                nc.vector.tensor_reduce(
                    dot, prod.rearrange("p (j m) -> p j m", m=M), AX.X, ALU.add
                )

                # normsq[p, j] = sum_m mem^2
                sq = spool.tile([P, J * M], F32, name=f"sq{b}", tag="sq")
                nc.scalar.activation(sq, mem_sb, AF.Square)
                nsq = smpool.tile([P, J], F32, name=f"nsq{b}", tag="nsq")
                nc.vector.tensor_reduce(
                    nsq, sq.rearrange("p (j m) -> p j m", m=M), AX.X, ALU.add
                )

                # sim = dot / (sqrt(nsq) * k_norm + EPS)
                mnorm = smpool.tile([P, J], F32, name=f"mnorm{b}", tag="mnorm")
                nc.scalar.activation(mnorm, nsq, AF.Sqrt)
                den = smpool.tile([P, J], F32, name=f"den{b}", tag="den")
                nc.vector.tensor_scalar(den, mnorm, kn_bc, EPS, ALU.mult, ALU.add)
                rden = smpool.tile([P, J], F32, name=f"rden{b}", tag="rden")
                nc.vector.reciprocal(rden, den)
                sim = smpool.tile([P, J], F32, name=f"sim{b}", tag="sim")
                nc.vector.tensor_tensor(sim, dot, rden, ALU.mult)

                # w_c = softmax(beta * sim) over all n (no max-sub: |beta*sim| small)
                e_t = smpool.tile([P, J], F32, name=f"e{b}", tag="e")
                esum = smpool.tile([P, 1], F32, name=f"esum{b}", tag="esum")
                nc.scalar.activation(e_t, sim, AF.Exp, scale=beta_bc, accum_out=esum)
                z_ps = ppool.tile([P, 1], F32, name=f"z_ps{b}", tag="scratch")
                nc.tensor.matmul(z_ps, ones128, esum)
                zr = smpool.tile([P, 1], F32, name=f"zr{b}", tag="zr")
                nc.vector.reciprocal(zr, z_ps)
                wc = smpool.tile([P, J], F32, name=f"wc{b}", tag="wc")
                nc.vector.tensor_scalar_mul(wc, e_t, zr)

                # w_g = g*w_c + (1-g)*w_prev
                t1 = smpool.tile([P, J], F32, name=f"t1{b}", tag="t1")
                nc.vector.tensor_scalar_mul(t1, wc, g_bc)
                t2 = smpool.tile([P, J], F32, name=f"t2{b}", tag="t2")
                nc.scalar.activation(
                    t2, wp_slab[:, b * J : (b + 1) * J], AF.Copy, scale=omg_bc
                )
                wg = smpool.tile([P, J], F32, name=f"wg{b}", tag="wg")
                nc.vector.tensor_tensor(wg, t1, t2, ALU.add)

                # circular rolls along n = 64p + j
                r1 = smpool.tile([P, J], F32, name=f"r1{b}", tag="r1")
                nc.vector.tensor_copy(r1[:, 1:J], wg[:, 0 : J - 1])
                c1_ps = ppool.tile([P, 1], F32, name=f"c1_ps{b}", tag="scratch")
                nc.tensor.matmul(c1_ps, sdn, wg[:, J - 1 : J])
                nc.vector.tensor_copy(r1[:, 0:1], c1_ps)

                rm1 = smpool.tile([P, J], F32, name=f"rm1{b}", tag="rm1")
                nc.vector.tensor_copy(rm1[:, 0 : J - 1], wg[:, 1:J])
                cm_ps = ppool.tile([P, 1], F32, name=f"cm_ps{b}", tag="scratch")
                nc.tensor.matmul(cm_ps, sup, wg[:, 0:1])
                nc.vector.tensor_copy(rm1[:, J - 1 : J], cm_ps)

                # w_s = s0*r1 + s1*wg + s2*rm1
                a1 = smpool.tile([P, J], F32, name=f"a1{b}", tag="a1")
                nc.vector.tensor_scalar_mul(a1, r1, s0_bc)
                a2 = smpool.tile([P, J], F32, name=f"a2{b}", tag="a2")
                nc.scalar.activation(a2, wg, AF.Copy, scale=s1_bc)
                a3 = smpool.tile([P, J], F32, name=f"a3{b}", tag="a3")
                nc.vector.tensor_scalar_mul(a3, rm1, s2_bc)
                t12 = smpool.tile([P, J], F32, name=f"t12{b}", tag="t12")
                nc.vector.tensor_tensor(t12, a1, a2, ALU.add)
                ws = smpool.tile([P, J], F32, name=f"ws{b}", tag="ws")
                nc.vector.tensor_tensor(ws, t12, a3, ALU.add)

                # w = ws^y / (sum + EPS)
                lnw = smpool.tile([P, J], F32, name=f"lnw{b}", tag="lnw")
                nc.scalar.activation(lnw, ws, AF.Ln)
                wpw = smpool.tile([P, J], F32, name=f"wpw{b}", tag="wpw")
                wps = smpool.tile([P, 1], F32, name=f"wps{b}", tag="wps")
                nc.scalar.activation(wpw, lnw, AF.Exp, scale=y_bc, accum_out=wps)
                zp_ps = ppool.tile([P, 1], F32, name=f"zp_ps{b}", tag="scratch")
                nc.tensor.matmul(zp_ps, ones128, wps)
                zpe = smpool.tile([P, 1], F32, name=f"zpe{b}", tag="zpe")
                nc.vector.tensor_scalar(zp_ps_sb := zpe, zp_ps, EPS, None, ALU.add)
                zpr = smpool.tile([P, 1], F32, name=f"zpr{b}", tag="zpr")
                nc.vector.reciprocal(zpr, zp_ps_sb)
                nc.vector.tensor_scalar_mul(w_slab[:, b * J : (b + 1) * J], wpw, zpr)

                # read: memory_data[b, m] = sum_n w[n] mem[n, m]
                md_ps = pmd.tile([1, M], F32, name=f"md_ps{b}", tag="md")
                for j in range(J):
                    nc.tensor.matmul(
                        md_ps,
                        w_slab[:, b * J + j : b * J + j + 1],
                        mem3[:, j, :],
                        start=(j == 0),
                        stop=(j == J - 1),
                    )
                nc.vector.tensor_copy(md_row[0:1, b * M : (b + 1) * M], md_ps)

            # ---------------- outputs ----------------
            nc.sync.dma_start(
                out=wout_d.rearrange("b (p j) -> p b j", p=P),
                in_=w_slab.rearrange("p (b j) -> p b j", b=B_LOC),
            )
            nc.sync.dma_start(out=md_d.rearrange("(a b) m -> a (b m)", a=1), in_=md_row)

    nc.finalize()
    return nc


def _get_nc():
    global _NC_CACHE
    if _NC_CACHE is None:
        _NC_CACHE = _build_nc()
    return _NC_CACHE


def kernel(**inputs):
    emb = np.ascontiguousarray(np.asarray(inputs["embeddings"], dtype=np.float32))
    wp = np.ascontiguousarray(np.asarray(inputs["w_prev"], dtype=np.float32))
    mem = np.ascontiguousarray(np.asarray(inputs["memory_weights"], dtype=np.float32))
    W = np.ascontiguousarray(np.asarray(inputs["W"], dtype=np.float32))
    bb = np.ascontiguousarray(np.asarray(inputs["b"], dtype=np.float32))

    nc = _get_nc()
    n_cores = 8
    in_maps = []
    for c in range(n_cores):
        sl = slice(c * B_LOC, (c + 1) * B_LOC)
        in_maps.append(
            {
                "embeddings": emb[sl],
                "w_prev": wp[sl],
                "memory_weights": mem[sl],
                "W": W,
                "b": bb,
            }
        )
    from concourse import bass_utils

    res = bass_utils.run_bass_kernel_spmd(nc, in_maps, list(range(n_cores)))
    md = np.concatenate([res.results[c]["memory_data"] for c in range(n_cores)], axis=0)
    w = np.concatenate([res.results[c]["w_out"] for c in range(n_cores)], axis=0)
    return md, w


# revision 21
# speedup vs baseline: 1.1822x; 1.1822x over previous
"""NTM read controller kernel for Trainium2 (8 NeuronCores, SPMD data-parallel over batch).

Problem shapes (hardcoded): B=64, E=512, N=8192, M=64, Dense out = M+6 = 70.
Sharding: batch 64 -> 8 cores x 8 batches. Each core is fully independent.

Per-core layout: n = 64*p + j  (partition p in [0,128), j in [0,64)).
memory_weights[b] loads as a [128, 4096] SBUF slab with 16 KiB contiguous per
partition (full-rate DMA).  Engine split:
  - DVE: dot-product mul + grouped reduces + small chain ops
  - ACT: square pass, Exp/Ln/Sqrt/Softplus/Sigmoid (scale = per-partition AP)
  - PE : controller matmul, partition-sum broadcast (all-ones matmul),
         circular-shift boundary columns (permutation matmul), weighted read
"""

import sys

for _p in ("/opt/trn_rl_repo", "/root/.axon_site/_ro/trn_rl_repo"):
    if _p not in sys.path:
        sys.path.insert(0, _p)

import numpy as np

import concourse.bass as bass
import concourse.bacc as bacc
import concourse.mybir as mybir
from concourse.tile import TileContext

F32 = mybir.dt.float32
F32R = mybir.dt.float32r
BF16 = mybir.dt.bfloat16
AF = mybir.ActivationFunctionType
ALU = mybir.AluOpType
AX = mybir.AxisListType

B_LOC = 8      # batches per core
E = 512
N = 8192
M = 64
C_OUT = 70     # M + 6
P = 128        # partitions
J = 64         # n = 64*p + j
EPS = 1e-8

_NC_CACHE = None


def _build_nc():
    nc = bacc.Bacc("TRN2", target_bir_lowering=False, debug=False, num_devices=8)

    emb_d = nc.dram_tensor("embeddings", [B_LOC, E], F32, kind="ExternalInput")
    wp_d = nc.dram_tensor("w_prev", [B_LOC, N], F32, kind="ExternalInput")
    mem_d = nc.dram_tensor("memory_weights", [B_LOC, N, M], F32, kind="ExternalInput")
    W_d = nc.dram_tensor("W", [E, C_OUT], F32, kind="ExternalInput")
    b_d = nc.dram_tensor("b", [C_OUT], F32, kind="ExternalInput")
    md_d = nc.dram_tensor("memory_data", [B_LOC, M], F32, kind="ExternalOutput")
    wout_d = nc.dram_tensor("w_out", [B_LOC, N], F32, kind="ExternalOutput")

    with TileContext(nc) as tc:
        with (
            tc.tile_pool(name="const", bufs=1) as cpool,
            tc.tile_pool(name="slab", bufs=2) as spool,
            tc.tile_pool(name="small", bufs=2) as smpool,
            tc.tile_pool(name="pscratch", bufs=2, space="PSUM") as ppool,
            tc.tile_pool(name="pmd", bufs=2, space="PSUM") as pmd,
        ):
            # ---------------- constants ----------------
            ones128 = cpool.tile([P, P], F32)
            nc.vector.memset(ones128, 1.0)

            id8 = cpool.tile([8, 8], F32)
            nc.vector.memset(id8, 1.0)
            nc.gpsimd.affine_select(
                id8, id8, pattern=[[-1, 8]], compare_op=ALU.is_equal,
                fill=0.0, base=0, channel_multiplier=1,
            )

            # roll(+1): out[p] = in[(p-1) mod 128]  -> Sdn[p, j]=1 iff j=p+1, patch [127,0]
            sdn = cpool.tile([P, P], F32)
            nc.vector.memset(sdn, 1.0)
            nc.gpsimd.affine_select(
                sdn, sdn, pattern=[[-1, P]], compare_op=ALU.is_equal,
                fill=0.0, base=1, channel_multiplier=1,
            )
            # wrap element (127, 0) via a second mask + add (engine APs must
            # start at partition 0/32/64/96, so no direct offset memset)
            sdn_wrap = cpool.tile([P, P], F32)
            nc.vector.memset(sdn_wrap, 1.0)
            nc.gpsimd.affine_select(
                sdn_wrap, sdn_wrap, pattern=[[-1, P]], compare_op=ALU.is_equal,
                fill=0.0, base=-(P - 1), channel_multiplier=1,
            )
            nc.vector.tensor_tensor(sdn, sdn, sdn_wrap, ALU.add)

            # roll(-1): out[p] = in[(p+1) mod 128]  -> Sup[p, j]=1 iff j=p-1, patch [0,127]
            sup = cpool.tile([P, P], F32)
            nc.vector.memset(sup, 1.0)
            nc.gpsimd.affine_select(
                sup, sup, pattern=[[-1, P]], compare_op=ALU.is_equal,
                fill=0.0, base=-1, channel_multiplier=1,
            )
            nc.vector.memset(sup[0:1, P - 1 : P], 1.0)

            ones18 = cpool.tile([1, 8], F32)
            nc.vector.memset(ones18, 1.0)

            # ---------------- small inputs ----------------
            W4 = cpool.tile([P, 4 * C_OUT], F32)
            nc.sync.dma_start(
                out=W4.rearrange("p (c j) -> p c j", c=4),
                in_=W_d.rearrange("(c p) j -> p c j", p=P),
            )
            b_sb = cpool.tile([1, C_OUT], F32)
            nc.sync.dma_start(out=b_sb, in_=b_d.rearrange("(a j) -> a j", a=1))

            emb_sb = cpool.tile([B_LOC, E], F32)
            nc.sync.dma_start(out=emb_sb, in_=emb_d[:, :])

            wp_slab = cpool.tile([P, B_LOC * J], F32)
            nc.sync.dma_start(
                out=wp_slab.rearrange("p (b j) -> p b j", b=B_LOC),
                in_=wp_d.rearrange("b (p j) -> p b j", p=P),
            )

            w_slab = cpool.tile([P, B_LOC * J], F32)   # final w output staging
            md_row = cpool.tile([1, B_LOC * M], F32)   # final memory_data staging
            md_gather = cpool.tile([1, B_LOC * 4 * M], F32)  # diag blocks staging

            # ---------------- controller: addr = emb @ W + b ----------------
            eT_sb = cpool.tile([P, 32], F32)  # 4 chunks of emb^T [128, 8]
            for c in range(4):
                eT_ps = ppool.tile([P, 8], F32, name=f"eT_ps{c}", tag="scratch")
                nc.tensor.matmul(
                    eT_ps, emb_sb[:, c * P : (c + 1) * P], id8, is_transpose=True
                )
                nc.vector.tensor_copy(eT_sb[:, c * 8 : (c + 1) * 8], eT_ps)

            addr_ps = ppool.tile([B_LOC, C_OUT], F32, tag="scratch")
            for c in range(4):
                nc.tensor.matmul(
                    addr_ps,
                    eT_sb[:, c * 8 : (c + 1) * 8],
                    W4.rearrange("p (c j) -> p c j", c=4)[:, c, :],
                    start=(c == 0),
                    stop=False,
                )
            nc.tensor.matmul(addr_ps, ones18, b_sb, start=False, stop=True)
            addr_sb = cpool.tile([B_LOC, C_OUT], F32)
            nc.vector.tensor_copy(addr_sb, addr_ps)

            # ---------------- controller nonlinearities ----------------
            # ctrl columns: 0=beta 1=g 2=1-g 3=y 4=k_norm 5..7=s
            ctrl = cpool.tile([B_LOC, 8], F32)
            sp_t = cpool.tile([B_LOC, 2], F32)  # softplus scratch: [beta_raw, y_raw]
            nc.scalar.activation(sp_t[:, 0:1], addr_sb[:, 64:65], AF.Exp)
            nc.scalar.activation(sp_t[:, 1:2], addr_sb[:, 69:70], AF.Exp)
            nc.vector.tensor_scalar(sp_t, sp_t, 1.0, None, ALU.add)
            nc.scalar.activation(ctrl[:, 0:1], sp_t[:, 0:1], AF.Ln)       # beta
            nc.scalar.activation(ctrl[:, 3:4], sp_t[:, 1:2], AF.Ln)      # softplus(y_raw)
            nc.scalar.activation(ctrl[:, 1:2], addr_sb[:, 65:66], AF.Sigmoid)
            nc.vector.tensor_scalar(
                ctrl[:, 2:3], ctrl[:, 1:2], -1.0, 1.0, ALU.mult, ALU.add
            )
            nc.vector.tensor_scalar(ctrl[:, 3:4], ctrl[:, 3:4], 1.0, None, ALU.add)

            ksq = cpool.tile([B_LOC, M], F32)
            knsq = cpool.tile([B_LOC, 1], F32)
            nc.scalar.activation(ksq, addr_sb[:, 0:M], AF.Square, accum_out=knsq)
            # sqrt(x) = exp(0.5*ln(x)) — stays in the exp/ln ACT table set
            knl = cpool.tile([B_LOC, 1], F32)
            nc.scalar.activation(knl, knsq, AF.Ln)
            nc.scalar.activation(ctrl[:, 4:5], knl, AF.Exp, scale=0.5)

            sexp = cpool.tile([B_LOC, 3], F32)
            ssum = cpool.tile([B_LOC, 1], F32)
            nc.scalar.activation(sexp, addr_sb[:, 66:69], AF.Exp, accum_out=ssum)
            srec = cpool.tile([B_LOC, 1], F32)
            nc.vector.reciprocal(srec, ssum)
            nc.vector.tensor_scalar_mul(ctrl[:, 5:8], sexp, srec)

            # ---------------- per-batch broadcast of controller scalars ----------------
            cb_list = []
            kb_list = []
            ones8p = cpool.tile([B_LOC, P], F32)
            nc.vector.memset(ones8p, 1.0)
            for b in range(B_LOC):
                # sel[p, :] = 1 iff p == b  (row-select mask, built from partition 0)
                sel = cpool.tile([B_LOC, P], F32, name=f"sel{b}")
                nc.gpsimd.affine_select(
                    sel, ones8p, pattern=[[0, P]], compare_op=ALU.is_equal,
                    fill=0.0, base=-b, channel_multiplier=1,
                )

                cb_ps = ppool.tile([P, 8], F32, name=f"cb_ps{b}", tag="scratch")
                nc.tensor.matmul(cb_ps, sel, ctrl)
                cb = cpool.tile([P, 8], F32, name=f"cb{b}")
                nc.vector.tensor_copy(cb, cb_ps)
                cb_list.append(cb)

                kb_ps = ppool.tile([P, M], F32, name=f"kb_ps{b}", tag="scratch")
                nc.tensor.matmul(kb_ps, sel, addr_sb[:, 0:M])
                kb = cpool.tile([P, M], F32, name=f"kb{b}")
                nc.vector.tensor_copy(kb, kb_ps)
                kb_list.append(kb)

            # ---------------- main per-batch loop ----------------
            for b in range(B_LOC):
                cb = cb_list[b]
                beta_bc = cb[:, 0:1]
                g_bc = cb[:, 1:2]
                omg_bc = cb[:, 2:3]
                y_bc = cb[:, 3:4]
                kn_bc = cb[:, 4:5]
                s0_bc = cb[:, 5:6]
                s1_bc = cb[:, 6:7]
                s2_bc = cb[:, 7:8]
                kb = kb_list[b]

                mem_sb = spool.tile([P, J * M], F32, name=f"mem{b}", tag="mem")
                nc.sync.dma_start(
                    out=mem_sb,
                    in_=mem_d[b].rearrange("(p j) m -> p (j m)", p=P),
                )
                mem3 = mem_sb.rearrange("p (j m) -> p j m", m=M)

                # dot[p, j] = sum_m mem[p, j, m] * k[m]
                prod = spool.tile([P, J * M], F32, name=f"prod{b}", tag="prod")
                kb3 = kb.unsqueeze(1).broadcast_to([P, J, M])
                # big elementwise mul on GpSimd — the only otherwise-idle engine
                nc.gpsimd.tensor_tensor(
                    prod.rearrange("p (j m) -> p j m", m=M), mem3, kb3, ALU.mult
                )
                dot = smpool.tile([P, J], F32, name=f"dot{b}", tag="dot")
                nc.vector.tensor_reduce(
                    dot, prod.rearrange("p (j m) -> p j m", m=M), AX.X, ALU.add
                )

                # bf16 copy of mem for the fast read matmuls (1 cyc/row on PE)
                memb = spool.tile([P, J * M], BF16, name=f"memb{b}", tag="memb")
                nc.scalar.activation(memb, mem_sb, AF.Copy)

                # normsq[p, j] = sum_m mem^2
                sq = spool.tile([P, J * M], F32, name=f"sq{b}", tag="sq")
                nc.scalar.activation(sq, mem_sb, AF.Square)
                nsq = smpool.tile([P, J], F32, name=f"nsq{b}", tag="nsq")
                nc.vector.tensor_reduce(
                    nsq, sq.rearrange("p (j m) -> p j m", m=M), AX.X, ALU.add
                )

                # sim = dot / (sqrt(nsq) * k_norm + EPS); sqrt via exp(0.5*ln)
                lnn = smpool.tile([P, J], F32, name=f"lnn{b}", tag="lnn")
                nc.scalar.activation(lnn, nsq, AF.Ln)
                mnorm = smpool.tile([P, J], F32, name=f"mnorm{b}", tag="mnorm")
                nc.scalar.activation(mnorm, lnn, AF.Exp, scale=0.5)
                den = smpool.tile([P, J], F32, name=f"den{b}", tag="den")
                nc.vector.tensor_scalar(den, mnorm, kn_bc, EPS, ALU.mult, ALU.add)
                rden = smpool.tile([P, J], F32, name=f"rden{b}", tag="rden")
                nc.vector.reciprocal(rden, den)
                sim = smpool.tile([P, J], F32, name=f"sim{b}", tag="sim")
                nc.vector.tensor_tensor(sim, dot, rden, ALU.mult)

                # w_c = softmax(beta * sim) over all n (no max-sub: |beta*sim| small)
                e_t = smpool.tile([P, J], F32, name=f"e{b}", tag="e")
                esum = smpool.tile([P, 1], F32, name=f"esum{b}", tag="esum")
                nc.scalar.activation(e_t, sim, AF.Exp, scale=beta_bc, accum_out=esum)
                z_ps = ppool.tile([P, 1], F32, name=f"z_ps{b}", tag="scratch")
                nc.tensor.matmul(z_ps, ones128, esum)
                zr = smpool.tile([P, 1], F32, name=f"zr{b}", tag="zr")
                nc.vector.reciprocal(zr, z_ps)
                wc = smpool.tile([P, J], F32, name=f"wc{b}", tag="wc")
                nc.vector.tensor_scalar_mul(wc, e_t, zr)

                # w_g = g*w_c + (1-g)*w_prev
                t1 = smpool.tile([P, J], F32, name=f"t1{b}", tag="t1")
                nc.vector.tensor_scalar_mul(t1, wc, g_bc)
                t2 = smpool.tile([P, J], F32, name=f"t2{b}", tag="t2")
                nc.scalar.activation(
                    t2, wp_slab[:, b * J : (b + 1) * J], AF.Copy, scale=omg_bc
                )
                wg = smpool.tile([P, J], F32, name=f"wg{b}", tag="wg")
                nc.vector.tensor_tensor(wg, t1, t2, ALU.add)

                # circular rolls along n = 64p + j
                r1 = smpool.tile([P, J], F32, name=f"r1{b}", tag="r1")
                nc.vector.tensor_copy(r1[:, 1:J], wg[:, 0 : J - 1])
                c1_ps = ppool.tile([P, 1], F32, name=f"c1_ps{b}", tag="scratch")
                nc.tensor.matmul(c1_ps, sdn, wg[:, J - 1 : J])
                nc.vector.tensor_copy(r1[:, 0:1], c1_ps)

                rm1 = smpool.tile([P, J], F32, name=f"rm1{b}", tag="rm1")
                nc.vector.tensor_copy(rm1[:, 0 : J - 1], wg[:, 1:J])
                cm_ps = ppool.tile([P, 1], F32, name=f"cm_ps{b}", tag="scratch")
                nc.tensor.matmul(cm_ps, sup, wg[:, 0:1])
                nc.vector.tensor_copy(rm1[:, J - 1 : J], cm_ps)

                # w_s = s0*r1 + s1*wg + s2*rm1
                a1 = smpool.tile([P, J], F32, name=f"a1{b}", tag="a1")
                nc.vector.tensor_scalar_mul(a1, r1, s0_bc)
                a2 = smpool.tile([P, J], F32, name=f"a2{b}", tag="a2")
                nc.scalar.activation(a2, wg, AF.Copy, scale=s1_bc)
                a3 = smpool.tile([P, J], F32, name=f"a3{b}", tag="a3")
                nc.vector.tensor_scalar_mul(a3, rm1, s2_bc)
                t12 = smpool.tile([P, J], F32, name=f"t12{b}", tag="t12")
                nc.vector.tensor_tensor(t12, a1, a2, ALU.add)
                ws = smpool.tile([P, J], F32, name=f"ws{b}", tag="ws")
                nc.vector.tensor_tensor(ws, t12, a3, ALU.add)

                # w = ws^y / (sum + EPS)
                lnw = smpool.tile([P, J], F32, name=f"lnw{b}", tag="lnw")
                nc.scalar.activation(lnw, ws, AF.Ln)
                wpw = smpool.tile([P, J], F32, name=f"wpw{b}", tag="wpw")
                wps = smpool.tile([P, 1], F32, name=f"wps{b}", tag="wps")
                nc.scalar.activation(wpw, lnw, AF.Exp, scale=y_bc, accum_out=wps)
                zp_ps = ppool.tile([P, 1], F32, name=f"zp_ps{b}", tag="scratch")
                nc.tensor.matmul(zp_ps, ones128, wps)
                zpe = smpool.tile([P, 1], F32, name=f"zpe{b}", tag="zpe")
                nc.vector.tensor_scalar(zp_ps_sb := zpe, zp_ps, EPS, None, ALU.add)
                zpr = smpool.tile([P, 1], F32, name=f"zpr{b}", tag="zpr")
                nc.vector.reciprocal(zpr, zp_ps_sb)
                nc.vector.tensor_scalar_mul(w_slab[:, b * J : (b + 1) * J], wpw, zpr)

                # read: memory_data[b, m] = sum_n w[n] mem[n, m]
                # quad-j float32r matmuls: lhsT = 4 w columns, rhs = 4 j-groups
                # (N=256 -> 1 cyc/row).  Wanted values are the diagonal blocks
                # of the [4, 256] PSUM accumulator; gather them per-batch with
                # 4 tiny DMAs into a one-partition staging row, reduce at end.
                wb_col = smpool.tile([P, J], BF16, name=f"wb_col{b}", tag="wb_col")
                nc.vector.tensor_copy(wb_col, w_slab[:, b * J : (b + 1) * J])
                md_ps = pmd.tile([4, 4 * M], F32, name=f"md_ps{b}", tag="md")
                for t in range(J // 4):
                    nc.tensor.matmul(
                        md_ps,
                        wb_col[:, 4 * t : 4 * t + 4],
                        memb[:, 4 * t * M : (4 * t + 4) * M],
                        start=(t == 0),
                        stop=(t == J // 4 - 1),
                    )
                md_sb4 = smpool.tile([4, 4 * M], F32, name=f"md_sb4{b}", tag="md_sb4")
                nc.vector.tensor_copy(md_sb4, md_ps)
                for i in range(4):
                    nc.sync.dma_start(
                        out=md_gather[0:1, b * 4 * M + i * M : b * 4 * M + (i + 1) * M],
                        in_=md_sb4[i : i + 1, i * M : (i + 1) * M],
                    )

            # ---------------- outputs ----------------
            # md[b, m] = sum_i md_gather[b, i, m]  (reduce over the 4 quad lanes)
            nc.vector.tensor_reduce(
                md_row.rearrange("a (b m) -> a b m", b=B_LOC),
                md_gather.rearrange("a (b i m) -> a b m i", b=B_LOC, i=4),
                AX.X,
                ALU.add,
            )
            nc.sync.dma_start(
                out=wout_d.rearrange("b (p j) -> p b j", p=P),
                in_=w_slab.rearrange("p (b j) -> p b j", b=B_LOC),
            )
            nc.sync.dma_start(out=md_d.rearrange("(a b) m -> a (b m)", a=1), in_=md_row)

    nc.finalize()
    return nc


def _get_nc():
    global _NC_CACHE
    if _NC_CACHE is None:
        _NC_CACHE = _build_nc()
    return _NC_CACHE


def kernel(**inputs):
    emb = np.ascontiguousarray(np.asarray(inputs["embeddings"], dtype=np.float32))
    wp = np.ascontiguousarray(np.asarray(inputs["w_prev"], dtype=np.float32))
    mem = np.ascontiguousarray(np.asarray(inputs["memory_weights"], dtype=np.float32))
    W = np.ascontiguousarray(np.asarray(inputs["W"], dtype=np.float32))
    bb = np.ascontiguousarray(np.asarray(inputs["b"], dtype=np.float32))

    nc = _get_nc()
    n_cores = 8
    in_maps = []
    for c in range(n_cores):
        sl = slice(c * B_LOC, (c + 1) * B_LOC)
        in_maps.append(
            {
                "embeddings": emb[sl],
                "w_prev": wp[sl],
                "memory_weights": mem[sl],
                "W": W,
                "b": bb,
            }
        )
    from concourse import bass_utils

    res = bass_utils.run_bass_kernel_spmd(nc, in_maps, list(range(n_cores)))
    md = np.concatenate([res.results[c]["memory_data"] for c in range(n_cores)], axis=0)
    w = np.concatenate([res.results[c]["w_out"] for c in range(n_cores)], axis=0)
    return md, w


# revision 27
# speedup vs baseline: 1.2582x; 1.0643x over previous
"""NTM read controller kernel for Trainium2 (8 NeuronCores, SPMD data-parallel over batch).

Problem shapes (hardcoded): B=64, E=512, N=8192, M=64, Dense out = M+6 = 70.
Sharding: batch 64 -> 8 cores x 8 batches. Each core is fully independent.

Per-core layout: n = 64*p + j  (partition p in [0,128), j in [0,64)).
memory_weights[b] loads as a [128, 4096] SBUF slab with 16 KiB contiguous per
partition (full-rate DMA).  Engine split:
  - DVE: dot-product mul + grouped reduces + small chain ops
  - ACT: square pass, Exp/Ln/Sqrt/Softplus/Sigmoid (scale = per-partition AP)
  - PE : controller matmul, partition-sum broadcast (all-ones matmul),
         circular-shift boundary columns (permutation matmul), weighted read
"""

import sys

for _p in ("/opt/trn_rl_repo", "/root/.axon_site/_ro/trn_rl_repo"):
    if _p not in sys.path:
        sys.path.insert(0, _p)

import numpy as np

import concourse.bass as bass
import concourse.bacc as bacc
import concourse.mybir as mybir
from concourse.tile import TileContext

F32 = mybir.dt.float32
F32R = mybir.dt.float32r
BF16 = mybir.dt.bfloat16
AF = mybir.ActivationFunctionType
ALU = mybir.AluOpType
AX = mybir.AxisListType

B_LOC = 8      # batches per core
E = 512
N = 8192
M = 64
C_OUT = 70     # M + 6
P = 128        # partitions
J = 64         # n = 64*p + j
EPS = 1e-8

_NC_CACHE = None


def _build_nc():
    nc = bacc.Bacc("TRN2", target_bir_lowering=False, debug=False, num_devices=8)

    emb_d = nc.dram_tensor("embeddings", [B_LOC, E], F32, kind="ExternalInput")
    wp_d = nc.dram_tensor("w_prev", [B_LOC, N], F32, kind="ExternalInput")
    mem_d = nc.dram_tensor("memory_weights", [B_LOC, N, M], F32, kind="ExternalInput")
    W_d = nc.dram_tensor("W", [E, C_OUT], F32, kind="ExternalInput")
    b_d = nc.dram_tensor("b", [C_OUT], F32, kind="ExternalInput")
    md_d = nc.dram_tensor("memory_data", [B_LOC, M], F32, kind="ExternalOutput")
    wout_d = nc.dram_tensor("w_out", [B_LOC, N], F32, kind="ExternalOutput")

    with TileContext(nc) as tc:
        with (
            tc.tile_pool(name="const", bufs=1) as cpool,
            tc.tile_pool(name="slab", bufs=2) as spool,
            tc.tile_pool(name="small", bufs=2) as smpool,
            tc.tile_pool(name="pscratch", bufs=2, space="PSUM") as ppool,
            tc.tile_pool(name="pmd", bufs=2, space="PSUM") as pmd,
        ):
            # ---------------- constants ----------------
            # Pre-load the one ACT table set covering every function this
            # kernel uses (ln/exp/square/copy) so the table-load pass never
            # has to switch sets inside the loop.
            _ltl = mybir.InstLoadActFuncSet(
                name=nc.get_next_instruction_name(), ins=[], outs=[]
            )
            _ltl.act_func_set_id = 6  # natural_log_exp_and_others
            nc.scalar.add_instruction(_ltl)

            ones128 = cpool.tile([P, P], F32)
            nc.vector.memset(ones128, 1.0)

            id8 = cpool.tile([8, 8], F32)
            nc.vector.memset(id8, 1.0)
            nc.gpsimd.affine_select(
                id8, id8, pattern=[[-1, 8]], compare_op=ALU.is_equal,
                fill=0.0, base=0, channel_multiplier=1,
            )

            # roll(+1): out[p] = in[(p-1) mod 128]  -> Sdn[p, j]=1 iff j=p+1, patch [127,0]
            sdn = cpool.tile([P, P], F32)
            nc.vector.memset(sdn, 1.0)
            nc.gpsimd.affine_select(
                sdn, sdn, pattern=[[-1, P]], compare_op=ALU.is_equal,
                fill=0.0, base=1, channel_multiplier=1,
            )
            # wrap element (127, 0) via a second mask + add (engine APs must
            # start at partition 0/32/64/96, so no direct offset memset)
            sdn_wrap = cpool.tile([P, P], F32)
            nc.vector.memset(sdn_wrap, 1.0)
            nc.gpsimd.affine_select(
                sdn_wrap, sdn_wrap, pattern=[[-1, P]], compare_op=ALU.is_equal,
                fill=0.0, base=-(P - 1), channel_multiplier=1,
            )
            nc.vector.tensor_tensor(sdn, sdn, sdn_wrap, ALU.add)

            # roll(-1): out[p] = in[(p+1) mod 128]  -> Sup[p, j]=1 iff j=p-1, patch [0,127]
            sup = cpool.tile([P, P], F32)
            nc.vector.memset(sup, 1.0)
            nc.gpsimd.affine_select(
                sup, sup, pattern=[[-1, P]], compare_op=ALU.is_equal,
                fill=0.0, base=-1, channel_multiplier=1,
            )
            nc.vector.memset(sup[0:1, P - 1 : P], 1.0)

            ones18 = cpool.tile([1, 8], F32)
            nc.vector.memset(ones18, 1.0)

            # ---------------- small inputs ----------------
            W4 = cpool.tile([P, 4 * C_OUT], F32)
            nc.sync.dma_start(
                out=W4.rearrange("p (c j) -> p c j", c=4),
                in_=W_d.rearrange("(c p) j -> p c j", p=P),
            )
            b_sb = cpool.tile([1, C_OUT], F32)
            nc.sync.dma_start(out=b_sb, in_=b_d.rearrange("(a j) -> a j", a=1))

            emb_sb = cpool.tile([B_LOC, E], F32)
            nc.sync.dma_start(out=emb_sb, in_=emb_d[:, :])

            wp_slab = cpool.tile([P, B_LOC * J], F32)
            nc.sync.dma_start(
                out=wp_slab.rearrange("p (b j) -> p b j", b=B_LOC),
                in_=wp_d.rearrange("b (p j) -> p b j", p=P),
            )

            w_slab = cpool.tile([P, B_LOC * J], F32)   # final w output staging
            md_row = cpool.tile([1, B_LOC * M], F32)   # final memory_data staging
            md_gather = cpool.tile([1, B_LOC * 8 * M], F32)  # diag blocks staging

            # ---------------- controller: addr = emb @ W + b ----------------
            eT_sb = cpool.tile([P, 32], F32)  # 4 chunks of emb^T [128, 8]
            for c in range(4):
                eT_ps = ppool.tile([P, 8], F32, name=f"eT_ps{c}", tag="scratch")
                nc.tensor.matmul(
                    eT_ps, emb_sb[:, c * P : (c + 1) * P], id8, is_transpose=True
                )
                nc.vector.tensor_copy(eT_sb[:, c * 8 : (c + 1) * 8], eT_ps)

            addr_ps = ppool.tile([B_LOC, C_OUT], F32, tag="scratch")
            for c in range(4):
                nc.tensor.matmul(
                    addr_ps,
                    eT_sb[:, c * 8 : (c + 1) * 8],
                    W4.rearrange("p (c j) -> p c j", c=4)[:, c, :],
                    start=(c == 0),
                    stop=False,
                )
            nc.tensor.matmul(addr_ps, ones18, b_sb, start=False, stop=True)
            addr_sb = cpool.tile([B_LOC, C_OUT], F32)
            nc.vector.tensor_copy(addr_sb, addr_ps)

            # ---------------- controller nonlinearities ----------------
            # ctrl columns: 0=beta 1=g 2=1-g 3=y 4=k_norm 5..7=s
            ctrl = cpool.tile([B_LOC, 8], F32)
            sp_t = cpool.tile([B_LOC, 2], F32)  # softplus scratch: [beta_raw, y_raw]
            nc.scalar.activation(sp_t[:, 0:1], addr_sb[:, 64:65], AF.Exp)
            nc.scalar.activation(sp_t[:, 1:2], addr_sb[:, 69:70], AF.Exp)
            nc.vector.tensor_scalar(sp_t, sp_t, 1.0, None, ALU.add)
            nc.scalar.activation(ctrl[:, 0:1], sp_t[:, 0:1], AF.Ln)       # beta
            nc.scalar.activation(ctrl[:, 3:4], sp_t[:, 1:2], AF.Ln)      # softplus(y_raw)
            # sigmoid via exp to stay inside the ln/exp table set:
            # g = 1/(1+exp(-x))
            emg = cpool.tile([B_LOC, 1], F32)
            nc.scalar.activation(emg, addr_sb[:, 65:66], AF.Exp, scale=-1.0)
            nc.vector.tensor_scalar(emg, emg, 1.0, None, ALU.add)
            nc.vector.reciprocal(ctrl[:, 1:2], emg)
            nc.vector.tensor_scalar(
                ctrl[:, 2:3], ctrl[:, 1:2], -1.0, 1.0, ALU.mult, ALU.add
            )
            nc.vector.tensor_scalar(ctrl[:, 3:4], ctrl[:, 3:4], 1.0, None, ALU.add)

            ksq = cpool.tile([B_LOC, M], F32)
            knsq = cpool.tile([B_LOC, 1], F32)
            nc.scalar.activation(ksq, addr_sb[:, 0:M], AF.Square, accum_out=knsq)
            # sqrt(x) = exp(0.5*ln(x)) — stays in the exp/ln ACT table set
            knl = cpool.tile([B_LOC, 1], F32)
            nc.scalar.activation(knl, knsq, AF.Ln)
            nc.scalar.activation(ctrl[:, 4:5], knl, AF.Exp, scale=0.5)

            sexp = cpool.tile([B_LOC, 3], F32)
            ssum = cpool.tile([B_LOC, 1], F32)
            nc.scalar.activation(sexp, addr_sb[:, 66:69], AF.Exp, accum_out=ssum)
            srec = cpool.tile([B_LOC, 1], F32)
            nc.vector.reciprocal(srec, ssum)
            nc.vector.tensor_scalar_mul(ctrl[:, 5:8], sexp, srec)

            # ---------------- per-batch broadcast of controller scalars ----------------
            cb_list = []
            kb_list = []
            ones8p = cpool.tile([B_LOC, P], F32)
            nc.vector.memset(ones8p, 1.0)
            for b in range(B_LOC):
                # sel[p, :] = 1 iff p == b  (row-select mask, built from partition 0)
                sel = cpool.tile([B_LOC, P], F32, name=f"sel{b}")
                nc.gpsimd.affine_select(
                    sel, ones8p, pattern=[[0, P]], compare_op=ALU.is_equal,
                    fill=0.0, base=-b, channel_multiplier=1,
                )

                cb_ps = ppool.tile([P, 8], F32, name=f"cb_ps{b}", tag="scratch")
                nc.tensor.matmul(cb_ps, sel, ctrl)
                cb = cpool.tile([P, 8], F32, name=f"cb{b}")
                nc.vector.tensor_copy(cb, cb_ps)
                cb_list.append(cb)

                kb_ps = ppool.tile([P, M], F32, name=f"kb_ps{b}", tag="scratch")
                nc.tensor.matmul(kb_ps, sel, addr_sb[:, 0:M])
                kb = cpool.tile([P, M], F32, name=f"kb{b}")
                nc.vector.tensor_copy(kb, kb_ps)
                kb_list.append(kb)

            # ---------------- main per-batch loop ----------------
            for b in range(B_LOC):
                cb = cb_list[b]
                beta_bc = cb[:, 0:1]
                g_bc = cb[:, 1:2]
                omg_bc = cb[:, 2:3]
                y_bc = cb[:, 3:4]
                kn_bc = cb[:, 4:5]
                s0_bc = cb[:, 5:6]
                s1_bc = cb[:, 6:7]
                s2_bc = cb[:, 7:8]
                kb = kb_list[b]

                mem_sb = spool.tile([P, J * M], F32, name=f"mem{b}", tag="mem")
                nc.sync.dma_start(
                    out=mem_sb,
                    in_=mem_d[b].rearrange("(p j) m -> p (j m)", p=P),
                )
                mem3 = mem_sb.rearrange("p (j m) -> p j m", m=M)

                # dot[p, j] = sum_m mem[p, j, m] * k[m]
                prod = spool.tile([P, J * M], F32, name=f"prod{b}", tag="prod")
                kb3 = kb.unsqueeze(1).broadcast_to([P, J, M])
                # big elementwise mul on GpSimd — the only otherwise-idle engine
                nc.gpsimd.tensor_tensor(
                    prod.rearrange("p (j m) -> p j m", m=M), mem3, kb3, ALU.mult
                )
                dot = smpool.tile([P, J], F32, name=f"dot{b}", tag="dot")
                nc.vector.tensor_reduce(
                    dot, prod.rearrange("p (j m) -> p j m", m=M), AX.X, ALU.add
                )

                # bf16 copy of mem for the fast read matmuls (1 cyc/row on PE)
                memb = spool.tile([P, J * M], BF16, name=f"memb{b}", tag="memb")
                nc.scalar.activation(memb, mem_sb, AF.Copy)

                # normsq[p, j] = sum_m mem^2
                sq = spool.tile([P, J * M], F32, name=f"sq{b}", tag="sq")
                nc.scalar.activation(sq, mem_sb, AF.Square)
                nsq = smpool.tile([P, J], F32, name=f"nsq{b}", tag="nsq")
                nc.vector.tensor_reduce(
                    nsq, sq.rearrange("p (j m) -> p j m", m=M), AX.X, ALU.add
                )

                # sim = dot / (sqrt(nsq) * k_norm + EPS); sqrt via exp(0.5*ln)
                lnn = smpool.tile([P, J], F32, name=f"lnn{b}", tag="lnn")
                nc.scalar.activation(lnn, nsq, AF.Ln)
                mnorm = smpool.tile([P, J], F32, name=f"mnorm{b}", tag="mnorm")
                nc.scalar.activation(mnorm, lnn, AF.Exp, scale=0.5)
                den = smpool.tile([P, J], F32, name=f"den{b}", tag="den")
                nc.vector.tensor_scalar(den, mnorm, kn_bc, EPS, ALU.mult, ALU.add)
                rden = smpool.tile([P, J], F32, name=f"rden{b}", tag="rden")
                nc.vector.reciprocal(rden, den)
                sim = smpool.tile([P, J], F32, name=f"sim{b}", tag="sim")
                nc.vector.tensor_tensor(sim, dot, rden, ALU.mult)

                # w_c = softmax(beta * sim) over all n (no max-sub: |beta*sim| small)
                e_t = smpool.tile([P, J], F32, name=f"e{b}", tag="e")
                esum = smpool.tile([P, 1], F32, name=f"esum{b}", tag="esum")
                nc.scalar.activation(e_t, sim, AF.Exp, scale=beta_bc, accum_out=esum)
                z_ps = ppool.tile([P, 1], F32, name=f"z_ps{b}", tag="scratch")
                nc.tensor.matmul(z_ps, ones128, esum)
                zr = smpool.tile([P, 1], F32, name=f"zr{b}", tag="zr")
                nc.vector.reciprocal(zr, z_ps)
                # fold softmax normalization and g together: t1 = e * (g/Z)
                gzr = smpool.tile([P, 1], F32, name=f"gzr{b}", tag="gzr")
                nc.vector.tensor_scalar_mul(gzr, zr, g_bc)

                # w_g = g*w_c + (1-g)*w_prev
                t1 = smpool.tile([P, J], F32, name=f"t1{b}", tag="t1")
                nc.vector.tensor_scalar_mul(t1, e_t, gzr)
                t2 = smpool.tile([P, J], F32, name=f"t2{b}", tag="t2")
                nc.scalar.activation(
                    t2, wp_slab[:, b * J : (b + 1) * J], AF.Copy, scale=omg_bc
                )
                wg = smpool.tile([P, J], F32, name=f"wg{b}", tag="wg")
                nc.vector.tensor_tensor(wg, t1, t2, ALU.add)

                # circular rolls along n = 64p + j
                r1 = smpool.tile([P, J], F32, name=f"r1{b}", tag="r1")
                nc.vector.tensor_copy(r1[:, 1:J], wg[:, 0 : J - 1])
                c1_ps = ppool.tile([P, 1], F32, name=f"c1_ps{b}", tag="scratch")
                nc.tensor.matmul(c1_ps, sdn, wg[:, J - 1 : J])
                nc.vector.tensor_copy(r1[:, 0:1], c1_ps)

                rm1 = smpool.tile([P, J], F32, name=f"rm1{b}", tag="rm1")
                nc.vector.tensor_copy(rm1[:, 0 : J - 1], wg[:, 1:J])
                cm_ps = ppool.tile([P, 1], F32, name=f"cm_ps{b}", tag="scratch")
                nc.tensor.matmul(cm_ps, sup, wg[:, 0:1])
                nc.vector.tensor_copy(rm1[:, J - 1 : J], cm_ps)

                # w_s = s0*r1 + s1*wg + s2*rm1
                a1 = smpool.tile([P, J], F32, name=f"a1{b}", tag="a1")
                nc.vector.tensor_scalar_mul(a1, r1, s0_bc)
                a2 = smpool.tile([P, J], F32, name=f"a2{b}", tag="a2")
                nc.scalar.activation(a2, wg, AF.Copy, scale=s1_bc)
                a3 = smpool.tile([P, J], F32, name=f"a3{b}", tag="a3")
                nc.vector.tensor_scalar_mul(a3, rm1, s2_bc)
                t12 = smpool.tile([P, J], F32, name=f"t12{b}", tag="t12")
                nc.vector.tensor_tensor(t12, a1, a2, ALU.add)
                ws = smpool.tile([P, J], F32, name=f"ws{b}", tag="ws")
                nc.vector.tensor_tensor(ws, t12, a3, ALU.add)

                # w = ws^y / (sum + EPS)
                lnw = smpool.tile([P, J], F32, name=f"lnw{b}", tag="lnw")
                nc.scalar.activation(lnw, ws, AF.Ln)
                wpw = smpool.tile([P, J], F32, name=f"wpw{b}", tag="wpw")
                wps = smpool.tile([P, 1], F32, name=f"wps{b}", tag="wps")
                nc.scalar.activation(wpw, lnw, AF.Exp, scale=y_bc, accum_out=wps)
                zp_ps = ppool.tile([P, 1], F32, name=f"zp_ps{b}", tag="scratch")
                nc.tensor.matmul(zp_ps, ones128, wps)
                zpe = smpool.tile([P, 1], F32, name=f"zpe{b}", tag="zpe")
                nc.vector.tensor_scalar(zp_ps_sb := zpe, zp_ps, EPS, None, ALU.add)
                zpr = smpool.tile([P, 1], F32, name=f"zpr{b}", tag="zpr")
                nc.vector.reciprocal(zpr, zp_ps_sb)
                nc.vector.tensor_scalar_mul(w_slab[:, b * J : (b + 1) * J], wpw, zpr)

                # read: memory_data[b, m] = sum_n w[n] mem[n, m]
                # quad-j float32r matmuls: lhsT = 4 w columns, rhs = 4 j-groups
                # (N=256 -> 1 cyc/row).  Wanted values are the diagonal blocks
                # of the [4, 256] PSUM accumulator; gather them per-batch with
                # 4 tiny DMAs into a one-partition staging row, reduce at end.
                wb_col = smpool.tile([P, J], BF16, name=f"wb_col{b}", tag="wb_col")
                nc.vector.tensor_copy(wb_col, w_slab[:, b * J : (b + 1) * J])
                md_ps = pmd.tile([8, 8 * M], F32, name=f"md_ps{b}", tag="md")
                for t in range(J // 8):
                    nc.tensor.matmul(
                        md_ps,
                        wb_col[:, 8 * t : 8 * t + 8],
                        memb[:, 8 * t * M : (8 * t + 8) * M],
                        start=(t == 0),
                        stop=(t == J // 8 - 1),
                    )
                md_sb8 = smpool.tile([8, 8 * M], F32, name=f"md_sb8{b}", tag="md_sb8")
                nc.vector.tensor_copy(md_sb8, md_ps)
                for i in range(8):
                    nc.sync.dma_start(
                        out=md_gather[0:1, b * 8 * M + i * M : b * 8 * M + (i + 1) * M],
                        in_=md_sb8[i : i + 1, i * M : (i + 1) * M],
                    )

            # ---------------- outputs ----------------
            # md[b, m] = sum_i md_gather[b, i, m]  (reduce over the 4 quad lanes)
            nc.vector.tensor_reduce(
                md_row.rearrange("a (b m) -> a b m", b=B_LOC),
                md_gather.rearrange("a (b i m) -> a b m i", b=B_LOC, i=8),
                AX.X,
                ALU.add,
            )
            nc.sync.dma_start(
                out=wout_d.rearrange("b (p j) -> p b j", p=P),
                in_=w_slab.rearrange("p (b j) -> p b j", b=B_LOC),
            )
            nc.sync.dma_start(out=md_d.rearrange("(a b) m -> a (b m)", a=1), in_=md_row)

    nc.finalize()
    return nc


def _get_nc():
    global _NC_CACHE
    if _NC_CACHE is None:
        _NC_CACHE = _build_nc()
    return _NC_CACHE


def kernel(**inputs):
    emb = np.ascontiguousarray(np.asarray(inputs["embeddings"], dtype=np.float32))
    wp = np.ascontiguousarray(np.asarray(inputs["w_prev"], dtype=np.float32))
    mem = np.ascontiguousarray(np.asarray(inputs["memory_weights"], dtype=np.float32))
    W = np.ascontiguousarray(np.asarray(inputs["W"], dtype=np.float32))
    bb = np.ascontiguousarray(np.asarray(inputs["b"], dtype=np.float32))

    nc = _get_nc()
    n_cores = 8
    in_maps = []
    for c in range(n_cores):
        sl = slice(c * B_LOC, (c + 1) * B_LOC)
        in_maps.append(
            {
                "embeddings": emb[sl],
                "w_prev": wp[sl],
                "memory_weights": mem[sl],
                "W": W,
                "b": bb,
            }
        )
    from concourse import bass_utils

    res = bass_utils.run_bass_kernel_spmd(nc, in_maps, list(range(n_cores)))
    md = np.concatenate([res.results[c]["memory_data"] for c in range(n_cores)], axis=0)
    w = np.concatenate([res.results[c]["w_out"] for c in range(n_cores)], axis=0)
    return md, w
